# revision 2
# baseline (speedup 1.0000x reference)
"""Trainium2 Bass kernel for a dense causal-attention transformer block.

Reference computation (fp32, B=2, S=2048, D=2048, H=16, HD=128):
    qkv = x @ Wqkv ; q,k,v split per head
    scores = (q @ k^T) * HD**-0.5, causal mask, softmax
    o = softmax(scores) @ v ; out = o @ Wo
Sharding: tensor-parallel over heads (4 groups of 4 heads) x data-parallel
over batch (2) = 8 cores. Each core computes a partial output projection
(its 512 o-channels x Wo rows); the host sums the 4 partials per batch.

Device layout tricks:
  - QKV + output projections run as residual-compensated fp8 DoubleRow
    matmuls: y = x8@w8 + xr8@w8 + x8@wr8 with x8/w8 the e4m3 quantizations
    and xr8/wr8 their (requantized) residuals. Each DoubleRow instruction
    contracts 2 k-planes of 128 at 0.5 PE cycles/row, so the 3-term sum
    costs 0.75x the bf16 cycles at ~bf16 accuracy (dropped xr*wr term is
    ~0.2% relative). Weights are host-prescaled x32 into e4m3's normal
    range; the PSUM->SBUF copies undo the scale for free on ACT.
  - Attention (scores/softmax/AV) stays bf16: its contraction depth per
    instruction is only 128, so DoubleRow would need zero-padding and
    direct fp8 q/k would cost ~5% relative error.
  - qT/kT are produced channels-on-partitions so score tiles come out
    TRANSPOSED [keys=128, queries=512]; softmax sum is then a matmul with
    an all-ones lhsT (no cross-partition reduce, no transposes anywhere).
  - No max-subtraction in softmax: scores ~ N(0,1), exp is safe in fp32,
    and masked entries are multiplied by 0 after exp.
  - HD**-0.5 scaling folded into Wq on the host; v carries x16 so the
    o-quantization for the fp8 projection lands in e4m3's normal range
    (undone by a 1/512 scale on the output stage copy).
"""

import numpy as np
import ml_dtypes

BF16 = ml_dtypes.bfloat16
E4 = ml_dtypes.float8_e4m3

B = 2
S = 2048
D = 2048
H = 16
HD = 128
P = 128
G = 4            # TP groups (heads per group = 4)
NH = H // G      # heads per core = 4
CH = NH * HD     # o-channels per core = 512
NJ = S // 512    # 4 S-chunks of 512
KK = D // P      # 16 contraction tiles
KP = KK // 2     # 8 DoubleRow k-plane pairs
ST = S // P      # 16 sequence row-tiles

WSCALE = 32.0    # host pre-scale on Wqkv/Wo into e4m3 normal range
VSCALE = 16.0    # extra scale carried by v so o16 quantizes well

_progs = {}

# normalizer strategy: "pe" = per-tile ones-matmuls on PE;
# "pair" = one DVE/GpSimd pairwise-add level, then half as many ones-matmuls
SUM_MODE = "pair"


def _build(repeat=1):
    """Build (once) the single-core Bass/Tile program shared by all 8 cores.

    repeat>1 executes the whole computation that many times inside one NEFF
    (used only for overhead-free timing via T(xN)-T(x1) differencing).
    """
    key = (repeat, SUM_MODE)
    if key in _progs:
        return _progs[key]

    import concourse.tile as tile
    from concourse import bacc, mybir

    f32 = mybir.dt.float32
    bf16 = mybir.dt.bfloat16
    fp8 = mybir.dt.float8e4
    EXP = mybir.ActivationFunctionType.Exp
    nc = bacc.Bacc("TRN2", target_bir_lowering=False, debug=False)

    # DRAM I/O, pre-packed on host so every DMA is contiguous per partition.
    # x8/xr: [p, nj, kk, q] = xT chunk layout (x[b].T tiled), e4m3 + residual
    # wq*/wk*: [p, mi, kk, m] (column-sharded Wqkv x32, q part x HD^-.5)
    # wv*: [p, kk, n]  (rhs layout, x32)
    # wo*: [p, h, ncol, n] (row-sharded Wo, x32)
    # masks: [k, j, q] binary causal masks for the 4 diagonal positions
    # out: [p, si, col] partial output (bf16; host sums in fp32)
    x8_d = nc.dram_tensor("x8", (P, NJ, KK, 512), fp8, kind="ExternalInput")
    xr_d = nc.dram_tensor("xr", (P, NJ, KK, 512), fp8, kind="ExternalInput")
    wq8_d = nc.dram_tensor("wq8", (P, NH, KK, P), fp8, kind="ExternalInput")
    wqr_d = nc.dram_tensor("wqr", (P, NH, KK, P), fp8, kind="ExternalInput")
    wk8_d = nc.dram_tensor("wk8", (P, NH, KK, P), fp8, kind="ExternalInput")
    wkr_d = nc.dram_tensor("wkr", (P, NH, KK, P), fp8, kind="ExternalInput")
    wv8_d = nc.dram_tensor("wv8", (P, KK, CH), fp8, kind="ExternalInput")
    wvr_d = nc.dram_tensor("wvr", (P, KK, CH), fp8, kind="ExternalInput")
    wo8_d = nc.dram_tensor("wo8", (P, NH, NJ, 512), fp8,
                           kind="ExternalInput")
    wor_d = nc.dram_tensor("wor", (P, NH, NJ, 512), fp8,
                           kind="ExternalInput")
    mask_d = nc.dram_tensor("masks", (P, NH, 512), bf16, kind="ExternalInput")
    out_d = nc.dram_tensor("out", (P, ST, D), bf16, kind="ExternalOutput")

    with tile.TileContext(nc) as tc:
        with (
            tc.tile_pool(name="persist", bufs=1) as pp,
            tc.tile_pool(name="psumA", bufs=2, space="PSUM") as psA,
            tc.tile_pool(name="psumW", bufs=2, space="PSUM") as psW,
            tc.tile_pool(name="psumB", bufs=1, space="PSUM") as psB,
        ):
            for rep in range(repeat):
                _emit_once(nc, tc, tile, mybir, pp, psA, psW, psB,
                           x8_d, xr_d, wq8_d, wqr_d, wk8_d, wkr_d,
                           wv8_d, wvr_d, wo8_d, wor_d, mask_d, out_d,
                           f32, bf16, fp8, EXP, rep)

    nc.compile()
    _progs[key] = nc
    return nc


def _emit_once(nc, tc, tile, mybir, pp, psA, psW, psB,
               x8_d, xr_d, wq8_d, wqr_d, wk8_d, wkr_d,
               wv8_d, wvr_d, wo8_d, wor_d, mask_d, out_d,
               f32, bf16, fp8, EXP, rep):
    DR = mybir.MatmulPerfMode.DoubleRow
    COPY = mybir.ActivationFunctionType.Copy
    r = f"r{rep}_"
    # wq/wk as per-head-group tiles so the first matmul group only depends
    # on a small DMA, not the whole weight
    wq8_t = [pp.tile([P, KK, P], fp8, name=f"{r}wq8{mi}", tag=f"wq8{mi}")
             for mi in range(NH)]
    wqr_t = [pp.tile([P, KK, P], fp8, name=f"{r}wqr{mi}", tag=f"wqr{mi}")
             for mi in range(NH)]
    wk8_t = [pp.tile([P, KK, P], fp8, name=f"{r}wk8{mi}", tag=f"wk8{mi}")
             for mi in range(NH)]
    wkr_t = [pp.tile([P, KK, P], fp8, name=f"{r}wkr{mi}", tag=f"wkr{mi}")
             for mi in range(NH)]
    # wv (phase 1) and wo (phase 3) share slots
    wv8_sb = pp.tile([P, KK, CH], fp8, name=r + "wv8_sb", tag="wvwo8")
    wvr_sb = pp.tile([P, KK, CH], fp8, name=r + "wvr_sb", tag="wvwor")
    q_sb = pp.tile([P, NH, S], bf16, name=r + "q_sb", tag="q")
    k_sb = pp.tile([P, NH, S], bf16, name=r + "k_sb", tag="k")
    v_sb = pp.tile([P, ST, CH], bf16, name=r + "v_sb", tag="v")
    o_sb = pp.tile([P, NH, S], bf16, name=r + "o_sb", tag="o")
    o8_sb = pp.tile([P, NH, S], fp8, name=r + "o8_sb", tag="o8")
    or8_sb = pp.tile([P, NH, S], fp8, name=r + "or8_sb", tag="or8")
    mask_sb = pp.tile([P, NH, 512], bf16, name=r + "mask_sb", tag="mask")
    ones_sb = pp.tile([P, P], bf16, name=r + "ones_sb", tag="ones")
    zbias = pp.tile([P, 1], f32, name=r + "zbias", tag="zbias")

    nc.gpsimd.memset(ones_sb[:], 1.0)
    nc.gpsimd.memset(zbias[:], 0.0)

    def dr3(pw, w8, wr8, x8, xr8, j, first, last):
        """One k-plane-pair step of the 3-term residual DoubleRow matmul.
        The two w8-lhsT instructions are adjacent to help weight reuse."""
        s = slice(2 * j, 2 * j + 2)
        nc.tensor.matmul(pw, w8[:, s], x8[:, s],
                         start=first, stop=False, perf_mode=DR)
        nc.tensor.matmul(pw, w8[:, s], xr8[:, s],
                         start=False, stop=False, perf_mode=DR)
        nc.tensor.matmul(pw, wr8[:, s], x8[:, s],
                         start=False, stop=last, perf_mode=DR)

    # ---------------- Phase 1: QKV projections ----------------
    with tc.tile_pool(name=r + "xpool", bufs=2) as xpool:
        xcs = {}
        # DMA issue order = arrival order: first x chunk (split in half)
        # and first weight slice land before everything else so PE can
        # start within a few us
        xcs[0] = (xpool.tile([P, KK, 512], fp8, name=f"{r}x8c0", tag="x8c"),
                  xpool.tile([P, KK, 512], fp8, name=f"{r}xrc0", tag="xrc"))
        nc.sync.dma_start(wq8_t[0][:], wq8_d[:, 0])
        for qtr in range(4):
            nc.sync.dma_start(
                xcs[0][0][:, qtr * KK // 4:(qtr + 1) * KK // 4],
                x8_d[:, 0, qtr * KK // 4:(qtr + 1) * KK // 4])
        nc.sync.dma_start(wqr_t[0][:], wqr_d[:, 0])
        for qtr in range(4):
            nc.sync.dma_start(
                xcs[0][1][:, qtr * KK // 4:(qtr + 1) * KK // 4],
                xr_d[:, 0, qtr * KK // 4:(qtr + 1) * KK // 4])
        for mi in range(1, NH):
            nc.sync.dma_start(wq8_t[mi][:], wq8_d[:, mi])
            nc.sync.dma_start(wqr_t[mi][:], wqr_d[:, mi])
        for mi in range(NH):
            nc.sync.dma_start(wk8_t[mi][:], wk8_d[:, mi])
            nc.sync.dma_start(wkr_t[mi][:], wkr_d[:, mi])
        nc.sync.dma_start(wv8_sb[:], wv8_d[:])
        nc.sync.dma_start(wvr_sb[:], wvr_d[:])
        nc.sync.dma_start(mask_sb[:], mask_d[:])

        for nj in range(NJ):
            xc = xcs.get(nj)
            if xc is None:
                xc = (xpool.tile([P, KK, 512], fp8, name=f"{r}x8c{nj}",
                                 tag="x8c"),
                      xpool.tile([P, KK, 512], fp8, name=f"{r}xrc{nj}",
                                 tag="xrc"))
                nc.sync.dma_start(xc[0][:], x8_d[:, nj])
                nc.sync.dma_start(xc[1][:], xr_d[:, nj])
            x8c, xrc = xc
            # qT, kT: [CH, S] channel-major (per head = 128 partitions).
            # Two 24-DR accumulation groups fill the two banks of one
            # 2-bank PSUM tile so a single ACT copy moves both out.
            for wp, dst, dn in (((wq8_t, wqr_t), q_sb, "q"),
                                ((wk8_t, wkr_t), k_sb, "k")):
                w8_t, wr_t = wp
                for mi0 in (0, 2):
                    pw = psW.tile([P, 1024], f32,
                                  name=f"{r}{dn}{nj}_{mi0}", tag="accW")
                    for half in (0, 1):
                        mi = mi0 + half
                        sl = pw[:, half * 512:(half + 1) * 512]
                        for j in range(KP):
                            dr3(sl, w8_t[mi], wr_t[mi], x8c, xrc, j,
                                j == 0, j == KP - 1)
                    nc.scalar.activation(
                        out=dst[:, mi0:mi0 + 2, nj * 512:(nj + 1) * 512],
                        in_=pw[:].rearrange("p (a b) -> p a b", a=2),
                        func=COPY, scale=1.0 / WSCALE)
            # v: [S, CH] row-major (keys on partitions); x tile is the
            # stationary operand here. v carries VSCALE for o-quantization.
            for si0 in (0, 2):
                pw = psW.tile([P, 1024], f32,
                              name=f"{r}v{nj}_{si0}", tag="accW")
                for half in (0, 1):
                    si = si0 + half
                    sl = pw[:, half * 512:(half + 1) * 512]
                    cs = slice(si * P, (si + 1) * P)
                    for j in range(KP):
                        s = slice(2 * j, 2 * j + 2)
                        nc.tensor.matmul(
                            sl, x8c[:, s, cs], wv8_sb[:, s],
                            start=(j == 0), stop=False, perf_mode=DR)
                        nc.tensor.matmul(
                            sl, xrc[:, s, cs], wv8_sb[:, s],
                            start=False, stop=False, perf_mode=DR)
                        nc.tensor.matmul(
                            sl, x8c[:, s, cs], wvr_sb[:, s],
                            start=False, stop=(j == KP - 1), perf_mode=DR)
                nc.scalar.activation(
                    out=v_sb[:, 4 * nj + si0:4 * nj + si0 + 2, :],
                    in_=pw[:].rearrange("p (a b) -> p a b", a=2),
                    func=COPY, scale=VSCALE / WSCALE)

    # wo reuses wv's slots (Tile serializes the DMA after last wv read)
    wo8_sb = pp.tile([P, NH, NJ, 512], fp8, name=r + "wo8_sb", tag="wvwo8")
    wor_sb = pp.tile([P, NH, NJ, 512], fp8, name=r + "wor_sb", tag="wvwor")
    nc.sync.dma_start(wo8_sb[:], wo8_d[:])
    nc.sync.dma_start(wor_sb[:], wor_d[:])

    # ---------- Phase 2+3: attention + output projection ----------
    ADD = mybir.AluOpType.add
    SUB = mybir.AluOpType.subtract
    eng_toggle = [0]

    with (
        tc.tile_pool(name=r + "apool", bufs=18) as apool,
        tc.tile_pool(name=r + "tpool", bufs=14) as tpool,
        tc.tile_pool(name=r + "rpool", bufs=3) as rpool,
        tc.tile_pool(name=r + "ostage", bufs=4) as ostage,
    ):
        def emit_A(qc, h):
            """scoresT [keys=128, queries=512], two key tiles per 2-bank
            PSUM tile so exp runs as one [128,1024] ACT op."""
            qs, qe = qc * 512, (qc + 1) * 512
            ktmax = 4 * qc + 4
            a_slices = []
            for kt0 in range(0, ktmax, 2):
                pw = psW.tile([P, 1024], f32,
                              name=f"{r}st{qc}_{h}_{kt0}", tag="accW")
                offs = (_diag_off(qc, kt0), _diag_off(qc, kt0 + 1))
                for j2 in (0, 1):
                    kt = kt0 + j2
                    # diagonal tiles: queries < 128j are fully masked —
                    # compute, exp, mask and consume only visible columns
                    off = offs[j2]
                    nc.tensor.matmul(
                        pw[:, j2 * 512 + off:(j2 + 1) * 512],
                        k_sb[:, h, kt * P:(kt + 1) * P],
                        q_sb[:, h, qs + off:qe], start=True, stop=True)
                a2 = apool.tile([P, 1024], bf16,
                                name=f"{r}a{qc}_{h}_{kt0}", tag="a")
                if offs == (0, 0):
                    nc.scalar.activation(a2[:], pw[:], EXP, bias=zbias[:])
                else:
                    for j2 in (0, 1):
                        off = offs[j2]
                        nc.scalar.activation(
                            a2[:, j2 * 512 + off:(j2 + 1) * 512],
                            pw[:, j2 * 512 + off:(j2 + 1) * 512],
                            EXP, bias=zbias[:])
                for j2 in (0, 1):
                    kt = kt0 + j2
                    sl = a2[:, j2 * 512:(j2 + 1) * 512]
                    if kt >= 4 * qc:  # diagonal tile: causal 0/1 mask
                        off = offs[j2]
                        nc.vector.tensor_mul(
                            out=sl[:, off:], in0=sl[:, off:],
                            in1=mask_sb[:, kt - 4 * qc, off:])
                    a_slices.append(sl)
            return a_slices

        def emit_B(qc, h, a_slices):
            """AV accumulation + normalizer + divide for one head."""
            qs, qe = qc * 512, (qc + 1) * 512
            ktmax = 4 * qc + 4
            po = psB.tile([P, 512], f32, name=f"{r}po{qc}_{h}", tag="po")
            for kt in range(ktmax):
                # same column restriction as the scores; kt=0 is always a
                # full-width write, so every po column is initialized by the
                # start=True matmul
                off = _diag_off(qc, kt)
                nc.tensor.matmul(
                    po[:, off:], v_sb[:, kt, h * HD:(h + 1) * HD],
                    a_slices[kt][:, off:],
                    start=(kt == 0), stop=(kt == ktmax - 1))
            # normalizer: column sums of a over all key tiles, replicated
            # to all partitions by the all-ones lhsT
            pn = psB.tile([P, 512], f32, name=f"{r}pn{qc}_{h}", tag="pn")
            # full tiles: pairwise-add on DVE/GpSimd halves the PE
            # sum-matmuls; diagonal tiles go in individually, restricted to
            # their visible columns. (off, rhs) list: full-width entry first
            # so the start=True matmul initializes every pn column.
            sum_rhs = []
            full = [a_slices[kt] for kt in range(ktmax)
                    if _diag_off(qc, kt) == 0 and kt < 4 * qc]
            diag = [(kt, _diag_off(qc, kt)) for kt in range(ktmax)
                    if kt >= 4 * qc]
            if SUM_MODE == "pair" and len(full) >= 2:
                for i in range(0, len(full) - 1, 2):
                    t = tpool.tile([P, 512], bf16,
                                   name=f"{r}ts{qc}_{h}_{i}", tag="tsum")
                    eng = (nc.vector if eng_toggle[0] % 2 == 0
                           else nc.gpsimd)
                    eng_toggle[0] += 1
                    eng.tensor_tensor(t[:], full[i], full[i + 1], ADD)
                    sum_rhs.append((0, t[:]))
                if len(full) % 2:
                    sum_rhs.append((0, full[-1]))
            else:
                sum_rhs = [(0, s) for s in full]
            sum_rhs += [(off, a_slices[kt][:, off:]) for kt, off in diag]
            for i, (off, t) in enumerate(sum_rhs):
                nc.tensor.matmul(pn[:, off:], ones_sb[:], t,
                                 start=(i == 0),
                                 stop=(i == len(sum_rhs) - 1))
            rec = rpool.tile([P, 512], f32, name=f"{r}rc{qc}_{h}",
                             tag="rec")
            nc.vector.reciprocal_approx_fast(rec[:], pn[:])
            # o16 = po * rec (v carries x16); fp8 + residual for the
            # projection's DoubleRow operands, produced on DVE/Pool
            nc.vector.tensor_mul(out=o_sb[:, h, qs:qe],
                                 in0=po[:], in1=rec[:])
            nc.gpsimd.tensor_copy(out=o8_sb[:, h, qs:qe],
                                  in_=o_sb[:, h, qs:qe])
            nc.gpsimd.tensor_tensor(out=or8_sb[:, h, qs:qe],
                                    in0=o_sb[:, h, qs:qe],
                                    in1=o8_sb[:, h, qs:qe], op=SUB)

        # software pipeline: at step t emit scores/exp for head-step t, the
        # AV/normalizer for step t-1 (its exps had a full step to finish),
        # and the projection for a chunk two steps after its last head
        steps = [(qc, h) for qc in range(NJ) for h in range(NH)]
        pend = None
        for t, (qc, h) in enumerate(steps):
            a = emit_A(qc, h)
            if pend is not None:
                emit_B(*pend)
            if t >= 2 and steps[t - 2][1] == NH - 1:
                _emit_proj(nc, psA, ostage, o8_sb, or8_sb, wo8_sb, wor_sb,
                           out_d, steps[t - 2][0], r)
            pend = (qc, h, a)
        emit_B(*pend)
        _emit_proj(nc, psA, ostage, o8_sb, or8_sb, wo8_sb, wor_sb,
                   out_d, NJ - 1, r)


def _diag_off(qc, kt):
    """First visible query column (within the 512 chunk) for key tile kt of
    chunk qc; 0 for fully-visible tiles."""
    if kt < 4 * qc:
        return 0
    return 128 * (kt - 4 * qc)


def _emit_proj(nc, psA, ostage, o8_sb, or8_sb, wo8_sb, wor_sb, out_d, qc, r):
    import concourse.mybir as mybir
    f32 = mybir.dt.float32
    bf16 = mybir.dt.bfloat16
    DR = mybir.MatmulPerfMode.DoubleRow
    for si in range(4 * qc, 4 * qc + 4):
        cs = slice(si * P, (si + 1) * P)
        for nc0 in (0, 2):
            # two column-block groups share one stage tile -> one DMA
            stg = ostage.tile([P, 1024], bf16,
                              name=f"{r}os{si}_{nc0}", tag="os")
            for half in (0, 1):
                ncol = nc0 + half
                acc = psA.tile([P, 512], f32,
                               name=f"{r}pr{si}_{ncol}", tag="accA")
                for h0 in (0, 2):
                    hs = slice(h0, h0 + 2)
                    first, last = h0 == 0, h0 == 2
                    nc.tensor.matmul(
                        acc[:], o8_sb[:, hs, cs], wo8_sb[:, hs, ncol],
                        start=first, stop=False, perf_mode=DR)
                    nc.tensor.matmul(
                        acc[:], o8_sb[:, hs, cs], wor_sb[:, hs, ncol],
                        start=False, stop=False, perf_mode=DR)
                    nc.tensor.matmul(
                        acc[:], or8_sb[:, hs, cs], wo8_sb[:, hs, ncol],
                        start=False, stop=last, perf_mode=DR)
                # undo WSCALE*VSCALE on the stage copy
                nc.vector.tensor_scalar_mul(
                    out=stg[:, half * 512:(half + 1) * 512], in0=acc[:],
                    scalar1=1.0 / (WSCALE * VSCALE))
            nc.sync.dma_start(
                out_d[:, si, nc0 * 512:(nc0 + 2) * 512], stg[:])


def _q8r(a):
    """e4m3 quantization + e4m3 residual of a float32 array."""
    a8 = a.astype(E4)
    r8 = (a - a8.astype(np.float32)).astype(E4)
    return a8, r8


def _pack_inputs(x, Wqkv, Wo):
    """Host-side shard + pack into the per-core DMA-friendly layouts.
    Arrays are shared between cores where identical (x per batch, weights
    per TP group, masks global)."""
    scale = np.float32(HD) ** np.float32(-0.5)
    masks = np.zeros((P, NH, 512), dtype=BF16)
    k_idx = np.arange(P)[:, None]
    q_idx = np.arange(512)[None, :]
    for j in range(NH):
        masks[:, j, :] = (P * j + k_idx <= q_idx).astype(BF16)

    xps = []
    for b in range(B):
        xb = np.asarray(x[b], dtype=np.float32)
        # xT packed: [p, nj, kk, q] with xT[128*kk+p, 512*nj+q] = xb[q', d']
        xt = np.ascontiguousarray(
            xb.reshape(NJ, 512, KK, P).transpose(3, 0, 2, 1))
        x8, xr = _q8r(xt)
        xps.append({"x8": np.ascontiguousarray(x8),
                    "xr": np.ascontiguousarray(xr)})

    wmaps = []
    for g in range(G):
        wq = (np.asarray(Wqkv[:, CH * g:CH * (g + 1)], np.float32)
              * scale * np.float32(WSCALE))
        wk = (np.asarray(Wqkv[:, D + CH * g:D + CH * (g + 1)], np.float32)
              * np.float32(WSCALE))
        wv = (np.asarray(Wqkv[:, 2 * D + CH * g:2 * D + CH * (g + 1)],
                         np.float32) * np.float32(WSCALE))
        wo = (np.asarray(Wo[CH * g:CH * (g + 1), :], np.float32)
              * np.float32(WSCALE))
        wqp = wq.reshape(KK, P, NH, P).transpose(1, 2, 0, 3)
        wkp = wk.reshape(KK, P, NH, P).transpose(1, 2, 0, 3)
        wvp = wv.reshape(KK, P, CH).transpose(1, 0, 2)
        wop = wo.reshape(NH, P, NJ, 512).transpose(1, 0, 2, 3)
        m = {}
        for nm, wp in (("wq", wqp), ("wk", wkp), ("wv", wvp), ("wo", wop)):
            w8, wr = _q8r(np.ascontiguousarray(wp))
            m[nm + "8"] = np.ascontiguousarray(w8)
            m[nm + "r"] = np.ascontiguousarray(wr)
        wmaps.append(m)

    return [{**xps[c // G], "masks": masks, **wmaps[c % G]}
            for c in range(8)]


def _unpack_outputs(results):
    """Sum the 4 TP partials per batch and restore [B, S, D]."""
    out = np.zeros((B, S, D), dtype=np.float32)
    for c, res in enumerate(results):
        b = c // G
        part = np.asarray(res["out"]).astype(np.float32)   # [p, si, col]
        out[b] += part.transpose(1, 0, 2).reshape(S, D)
    return out


def kernel(x, Wqkv, Wo, _trace=False, _trace_kwargs=None):
    from concourse import bass_utils

    nc = _build()
    in_maps = _pack_inputs(x, Wqkv, Wo)
    res = bass_utils.run_bass_kernel_spmd(
        nc, in_maps, core_ids=list(range(8)), trace=_trace,
        **(_trace_kwargs or {}))
    out = _unpack_outputs(res.results)
    if _trace:
        kernel.last_result = res
    return out


# revision 17
# speedup vs baseline: 1.4870x; 1.4870x over previous
"""Trainium2 Bass kernel for a dense causal-attention transformer block.

Reference computation (fp32, B=2, S=2048, D=2048, H=16, HD=128):
    qkv = x @ Wqkv ; q,k,v split per head
    scores = (q @ k^T) * HD**-0.5, causal mask, softmax
    o = softmax(scores) @ v ; out = o @ Wo

Sharding: tensor-parallel over heads (4 groups of 4 heads) x data-parallel
over batch (2) = 8 cores. Each core computes a partial output projection
(its 512 o-channels x Wo rows); the host sums the 4 partials per batch.

Device layout tricks:
  - All matmul inputs are bf16 (4x faster PE than fp32); PSUM accum fp32.
  - qT/kT are produced channels-on-partitions so score tiles come out
    TRANSPOSED [keys=128, queries=512]; softmax sum is then a matmul with
    an all-ones lhsT (no cross-partition reduce, no transposes anywhere).
  - No max-subtraction in softmax: scores ~ N(0,1), exp is safe in fp32,
    and masked entries are multiplied by 0 after exp.
  - HD**-0.5 scaling folded into Wq on the host.
"""

import numpy as np
import ml_dtypes

BF16 = ml_dtypes.bfloat16

B = 2
S = 2048
D = 2048
H = 16
HD = 128
P = 128
G = 4            # TP groups (heads per group = 4)
NH = H // G      # heads per core = 4
CH = NH * HD     # o-channels per core = 512
NJ = S // 512    # 4 S-chunks of 512
KK = D // P      # 16 contraction tiles
ST = S // P      # 16 sequence row-tiles

_progs = {}

# normalizer strategy: "pe" = per-tile ones-matmuls on PE;
# "pair" = one DVE/GpSimd pairwise-add level, then half as many ones-matmuls
# "quad" = two fold levels, quartering the ones-matmuls for full tiles
SUM_MODE = "quad"


def _build(repeat=1):
    """Build (once) the single-core Bass/Tile program shared by all 8 cores.

    repeat>1 executes the whole computation that many times inside one NEFF
    (used only for overhead-free timing via T(xN)-T(x1) differencing).
    """
    key = (repeat, SUM_MODE)
    if key in _progs:
        return _progs[key]

    import concourse.tile as tile
    from concourse import bacc, mybir

    f32 = mybir.dt.float32
    bf16 = mybir.dt.bfloat16
    EXP = mybir.ActivationFunctionType.Exp

    nc = bacc.Bacc("TRN2", target_bir_lowering=False, debug=False)

    # DRAM I/O, pre-packed on host so every DMA is contiguous per partition.
    # x:  [p, nj, kk, q]  = xT chunk layout (x[b].T tiled)
    # wq/wk: [p, mi, kk, m] (column-sharded Wqkv, q part prescaled by HD^-.5)
    # wv: [p, kk, n]      (rhs layout)
    # wo: [p, h, ncol, n] (row-sharded Wo)
    # masks: [k, j, q]    binary causal masks for the 4 diagonal positions
    # out: [p, si, col]   partial output (fp32)
    x_d = nc.dram_tensor("x", (P, NJ, KK, 512), bf16, kind="ExternalInput")
    wq_d = nc.dram_tensor("wq", (P, NH, KK, P), bf16, kind="ExternalInput")
    wk_d = nc.dram_tensor("wk", (P, NH, KK, P), bf16, kind="ExternalInput")
    wv_d = nc.dram_tensor("wv", (P, KK, CH), bf16, kind="ExternalInput")
    wo_d = nc.dram_tensor("wo", (P, NH, NJ, 512), bf16, kind="ExternalInput")
    mask_d = nc.dram_tensor("masks", (P, NH, 512), bf16, kind="ExternalInput")
    # partial outputs in bf16 (halves output DMA); host sums them in fp32
    out_d = nc.dram_tensor("out", (P, ST, D), bf16, kind="ExternalOutput")

    with tile.TileContext(nc) as tc:
        with (
            tc.tile_pool(name="persist", bufs=1) as pp,
            tc.tile_pool(name="psumA", bufs=2, space="PSUM") as psA,
            tc.tile_pool(name="psumW", bufs=2, space="PSUM") as psW,
            tc.tile_pool(name="psumB", bufs=1, space="PSUM") as psB,
        ):
            for rep in range(repeat):
                _emit_once(nc, tc, tile, mybir, pp, psA, psW, psB,
                           x_d, wq_d, wk_d, wv_d, wo_d, mask_d, out_d,
                           f32, bf16, EXP, rep)

    nc.compile()
    _progs[key] = nc
    return nc


def _emit_once(nc, tc, tile, mybir, pp, psA, psW, psB,
               x_d, wq_d, wk_d, wv_d, wo_d, mask_d, out_d,
               f32, bf16, EXP, rep):
    r = f"r{rep}_"
    # wq/wk as 4 per-head-group tiles so the first matmul group only
    # depends on a 0.5MB DMA, not the whole weight
    wq_t = [pp.tile([P, KK, P], bf16, name=f"{r}wq{mi}", tag=f"wq{mi}")
            for mi in range(NH)]
    wk_t = [pp.tile([P, KK, P], bf16, name=f"{r}wk{mi}", tag=f"wk{mi}")
            for mi in range(NH)]
    # wv (phase 1) and wo (phase 3) share one 16KB slot
    wv_sb = pp.tile([P, KK, CH], bf16, name=r + "wv_sb", tag="wvwo")
    q_sb = pp.tile([P, NH, S], bf16, name=r + "q_sb", tag="q")
    k_sb = pp.tile([P, NH, S], bf16, name=r + "k_sb", tag="k")
    v_sb = pp.tile([P, ST, CH], bf16, name=r + "v_sb", tag="v")
    o_sb = pp.tile([P, NH, S], bf16, name=r + "o_sb", tag="o")
    mask_sb = pp.tile([P, NH, 512], bf16, name=r + "mask_sb", tag="mask")
    ones_sb = pp.tile([P, P], bf16, name=r + "ones_sb", tag="ones")
    zbias = pp.tile([P, 1], f32, name=r + "zbias", tag="zbias")

    nc.gpsimd.memset(ones_sb[:], 1.0)
    nc.gpsimd.memset(zbias[:], 0.0)

    # ---- Phase 1 + 2 interleaved: QKV projections + attention ----
    ADD = mybir.AluOpType.add
    eng_toggle = [0]

    with (
        tc.tile_pool(name=r + "apool", bufs=18) as apool,
        tc.tile_pool(name=r + "tpool", bufs=14) as tpool,
        tc.tile_pool(name=r + "rpool", bufs=3) as rpool,
    ):
        def qkv_thunks(nj, xc):
            """6 thunks, one per 2-bank accumulation group: q, k pairs of
            head-tiles then v pairs of seq-tiles. qT/kT land channel-major
            (per head = 128 partitions); a single ACT copy moves both banks
            out."""
            def qk(w_t, dst, dn, mi0):
                pw = psW.tile([P, 1024], f32,
                              name=f"{r}{dn}{nj}_{mi0}", tag="accW")
                for half in (0, 1):
                    mi = mi0 + half
                    for kk in range(KK):
                        nc.tensor.matmul(
                            pw[:, half * 512:(half + 1) * 512],
                            w_t[mi][:, kk, :], xc[:, kk, :],
                            start=(kk == 0), stop=(kk == KK - 1))
                nc.scalar.copy(
                    out=dst[:, mi0:mi0 + 2, nj * 512:(nj + 1) * 512],
                    in_=pw[:].rearrange("p (a b) -> p a b", a=2))

            def v(si0):
                # v: [S, CH] row-major (keys on partitions), same pairing
                pw = psW.tile([P, 1024], f32,
                              name=f"{r}v{nj}_{si0}", tag="accW")
                for half in (0, 1):
                    si = si0 + half
                    for kk in range(KK):
                        nc.tensor.matmul(
                            pw[:, half * 512:(half + 1) * 512],
                            xc[:, kk, si * P:(si + 1) * P],
                            wv_sb[:, kk, :],
                            start=(kk == 0), stop=(kk == KK - 1))
                nc.scalar.copy(
                    out=v_sb[:, 4 * nj + si0:4 * nj + si0 + 2, :],
                    in_=pw[:].rearrange("p (a b) -> p a b", a=2))

            import functools
            return [functools.partial(qk, wq_t, q_sb, "q", 0),
                    functools.partial(qk, wq_t, q_sb, "q", 2),
                    functools.partial(qk, wk_t, k_sb, "k", 0),
                    functools.partial(qk, wk_t, k_sb, "k", 2),
                    functools.partial(v, 0),
                    functools.partial(v, 2)]

        def emit_A(qc, h, use_psw=False):
            """scoresT [keys=128, queries=512]. Woven into QKV (use_psw
            False): single-bank tiles from the psA ring, which is free
            until the projection, so QKV keeps the 2-slot psW ring to
            itself. Woven into the projection (use_psw True): two key
            tiles per 2-bank psW tile — psW is the free ring there."""
            qs, qe = qc * 512, (qc + 1) * 512
            ktmax = 4 * qc + 4
            a_slices = []
            for kt0 in range(0, ktmax, 2):
                a2 = apool.tile([P, 1024], bf16,
                                name=f"{r}a{qc}_{h}_{kt0}", tag="a")
                pw2 = (psW.tile([P, 1024], f32,
                                name=f"{r}st{qc}_{h}_{kt0}", tag="accW")
                       if use_psw else None)
                for j2 in (0, 1):
                    kt = kt0 + j2
                    # diagonal tiles: queries < 128j are fully masked —
                    # compute, exp, mask and consume only visible columns
                    off = _diag_off(qc, kt)
                    if use_psw:
                        pw = pw2[:, j2 * 512:(j2 + 1) * 512]
                    else:
                        pw = psA.tile([P, 512], f32,
                                      name=f"{r}st{qc}_{h}_{kt}",
                                      tag="accA")[:]
                    nc.tensor.matmul(
                        pw[:, off:],
                        k_sb[:, h, kt * P:(kt + 1) * P],
                        q_sb[:, h, qs + off:qe], start=True, stop=True)
                    nc.scalar.activation(
                        a2[:, j2 * 512 + off:(j2 + 1) * 512],
                        pw[:, off:], EXP, bias=zbias[:])
                    sl = a2[:, j2 * 512:(j2 + 1) * 512]
                    if kt >= 4 * qc:  # diagonal tile: causal 0/1 mask
                        nc.vector.tensor_mul(
                            out=sl[:, off:], in0=sl[:, off:],
                            in1=mask_sb[:, kt - 4 * qc, off:])
                    a_slices.append(sl)
            return a_slices

        def emit_B(qc, h, a_slices):
            """AV accumulation + normalizer + divide for one head."""
            qs, qe = qc * 512, (qc + 1) * 512
            ktmax = 4 * qc + 4
            po = psB.tile([P, 512], f32, name=f"{r}po{qc}_{h}", tag="po")
            for kt in range(ktmax):
                # same column restriction as the scores; kt=0 is always a
                # full-width write, so every po column is initialized by the
                # start=True matmul
                off = _diag_off(qc, kt)
                nc.tensor.matmul(
                    po[:, off:], v_sb[:, kt, h * HD:(h + 1) * HD],
                    a_slices[kt][:, off:],
                    start=(kt == 0), stop=(kt == ktmax - 1))
            # normalizer: column sums of a over all key tiles, replicated
            # to all partitions by the all-ones lhsT
            pn = psB.tile([P, 512], f32, name=f"{r}pn{qc}_{h}", tag="pn")
            # full tiles: pairwise-add on DVE/GpSimd halves the PE
            # sum-matmuls; diagonal tiles go in individually, restricted to
            # their visible columns. (off, rhs) list: full-width entry first
            # so the start=True matmul initializes every pn column.
            sum_rhs = []
            full = [a_slices[kt] for kt in range(ktmax)
                    if _diag_off(qc, kt) == 0 and kt < 4 * qc]
            diag = [(kt, _diag_off(qc, kt)) for kt in range(ktmax)
                    if kt >= 4 * qc]
            if SUM_MODE in ("pair", "quad") and len(full) >= 2:
                lvl = list(full)
                nfold = 1 if SUM_MODE == "pair" else 2
                for fold in range(nfold):
                    if len(lvl) < 2:
                        break
                    nxt = []
                    for i in range(0, len(lvl) - 1, 2):
                        t = tpool.tile([P, 512], bf16,
                                       name=f"{r}ts{qc}_{h}_{fold}_{i}",
                                       tag="tsum")
                        eng = (nc.vector if eng_toggle[0] % 2 == 0
                               else nc.gpsimd)
                        eng_toggle[0] += 1
                        eng.tensor_tensor(t[:], lvl[i], lvl[i + 1], ADD)
                        nxt.append(t[:])
                    if len(lvl) % 2:
                        nxt.append(lvl[-1])
                    lvl = nxt
                sum_rhs = [(0, t) for t in lvl]
            else:
                sum_rhs = [(0, s) for s in full]
            sum_rhs += [(off, a_slices[kt][:, off:]) for kt, off in diag]
            for i, (off, t) in enumerate(sum_rhs):
                nc.tensor.matmul(pn[:, off:], ones_sb[:], t,
                                 start=(i == 0),
                                 stop=(i == len(sum_rhs) - 1))
            rec = rpool.tile([P, 512], f32, name=f"{r}rc{qc}_{h}",
                             tag="rec")
            nc.vector.reciprocal_approx_fast(rec[:], pn[:])
            nc.vector.tensor_mul(out=o_sb[:, h, qs:qe],
                                 in0=po[:], in1=rec[:])

        xpool_cm = tc.tile_pool(name=r + "xpool", bufs=2)
        xpool = xpool_cm.__enter__()
        # DMA issue order = arrival order: first x chunk (split in half)
        # and first weight slice land before everything else so PE can
        # start within a few us
        xcs = {0: xpool.tile([P, KK, 512], bf16, name=f"{r}xc0", tag="xc")}
        nc.sync.dma_start(wq_t[0][:], wq_d[:, 0])
        for qtr in range(4):
            nc.sync.dma_start(
                xcs[0][:, qtr * KK // 4:(qtr + 1) * KK // 4],
                x_d[:, 0, qtr * KK // 4:(qtr + 1) * KK // 4])
        for mi in range(1, NH):
            nc.sync.dma_start(wq_t[mi][:], wq_d[:, mi])
        for mi in range(NH):
            nc.sync.dma_start(wk_t[mi][:], wk_d[:, mi])
        nc.sync.dma_start(wv_sb[:], wv_d[:])
        nc.sync.dma_start(mask_sb[:], mask_d[:])

        # software pipeline: chunk nj's QKV groups woven with chunk nj-1's
        # attention head-steps (their q/k/v landed a full chunk ago, so no
        # copy-latency stalls); emit_B for a step runs one head-step after
        # its emit_A so the exps have time to finish. The last chunk's
        # steps run back-to-back after its QKV.
        pend = [None]

        def attn(qc, h, use_psw=False):
            a = emit_A(qc, h, use_psw)
            if pend[0] is not None:
                emit_B(*pend[0])
            pend[0] = (qc, h, a)

        for nj in range(NJ):
            xc = xcs.get(nj)
            if xc is None:
                xc = xpool.tile([P, KK, 512], bf16, name=f"{r}xc{nj}",
                                tag="xc")
                nc.sync.dma_start(xc[:], x_d[:, nj])
            for i, thunk in enumerate(qkv_thunks(nj, xc)):
                thunk()
                if nj > 0 and 1 <= i <= NH:
                    attn(nj - 1, i - 1)
        xpool_cm.__exit__(None, None, None)

        # ------- Phase 3: last chunk's attention woven with the -------
        # ------- output projection (scores now on the psW ring) -------
        # wo reuses wv's slot (Tile starts the DMA once nj=3's v groups
        # finish; the ~15us of qc=3 attention ahead of proj(0) hides it)
        wo_sb = pp.tile([P, NH, NJ, 512], bf16, name=r + "wo_sb",
                        tag="wvwo")
        nc.sync.dma_start(wo_sb[:], wo_d[:])
        with tc.tile_pool(name=r + "ostage", bufs=4) as ostage:
            for h in range(NH):
                attn(NJ - 1, h, use_psw=True)
                if h > 0:
                    _emit_proj(nc, psA, ostage, o_sb, wo_sb, out_d,
                               h - 1, r)
            emit_B(*pend[0])
            _emit_proj(nc, psA, ostage, o_sb, wo_sb, out_d, NJ - 1, r)


def _diag_off(qc, kt):
    """First visible query column (within the 512 chunk) for key tile kt of
    chunk qc; 0 for fully-visible tiles."""
    if kt < 4 * qc:
        return 0
    return 128 * (kt - 4 * qc)


def _emit_proj(nc, psA, ostage, o_sb, wo_sb, out_d, qc, r):
    import concourse.mybir as mybir
    f32 = mybir.dt.float32
    bf16 = mybir.dt.bfloat16
    for si in range(4 * qc, 4 * qc + 4):
        for nc0 in (0, 2):
            # two column-block groups share one stage tile -> one DMA
            stg = ostage.tile([P, 1024], bf16,
                              name=f"{r}os{si}_{nc0}", tag="os")
            for half in (0, 1):
                ncol = nc0 + half
                acc = psA.tile([P, 512], f32,
                               name=f"{r}pr{si}_{ncol}", tag="accA")
                for h in range(NH):
                    nc.tensor.matmul(
                        acc[:], o_sb[:, h, si * P:(si + 1) * P],
                        wo_sb[:, h, ncol, :],
                        start=(h == 0), stop=(h == NH - 1))
                nc.vector.tensor_copy(
                    out=stg[:, half * 512:(half + 1) * 512], in_=acc[:])
            nc.sync.dma_start(
                out_d[:, si, nc0 * 512:(nc0 + 2) * 512], stg[:])


def _pack_inputs(x, Wqkv, Wo):
    """Host-side shard + pack into the per-core DMA-friendly layouts.
    Arrays are shared between cores where identical (x per batch, weights
    per TP group, masks global)."""
    scale = np.float32(HD) ** np.float32(-0.5)
    masks = np.zeros((P, NH, 512), dtype=BF16)
    k_idx = np.arange(P)[:, None]
    q_idx = np.arange(512)[None, :]
    for j in range(NH):
        masks[:, j, :] = (P * j + k_idx <= q_idx).astype(BF16)

    xps = []
    for b in range(B):
        xb = np.asarray(x[b], dtype=np.float32)
        # xT packed: [p, nj, kk, q] with xT[128*kk+p, 512*nj+q] = xb[q', d']
        xps.append(np.ascontiguousarray(
            xb.astype(BF16).reshape(NJ, 512, KK, P).transpose(3, 0, 2, 1)))

    wmaps = []
    for g in range(G):
        wq = (np.asarray(Wqkv[:, CH * g:CH * (g + 1)], np.float32) * scale)
        wk = np.asarray(Wqkv[:, D + CH * g:D + CH * (g + 1)], np.float32)
        wv = np.asarray(Wqkv[:, 2 * D + CH * g:2 * D + CH * (g + 1)],
                        np.float32)
        wo = np.asarray(Wo[CH * g:CH * (g + 1), :], np.float32)
        wmaps.append({
            "wq": np.ascontiguousarray(
                wq.astype(BF16).reshape(KK, P, NH, P).transpose(1, 2, 0, 3)),
            "wk": np.ascontiguousarray(
                wk.astype(BF16).reshape(KK, P, NH, P).transpose(1, 2, 0, 3)),
            "wv": np.ascontiguousarray(
                wv.astype(BF16).reshape(KK, P, CH).transpose(1, 0, 2)),
            "wo": np.ascontiguousarray(
                wo.astype(BF16).reshape(NH, P, NJ, 512).transpose(1, 0, 2, 3)),
        })

    return [{"x": xps[c // G], "masks": masks, **wmaps[c % G]}
            for c in range(8)]


def _unpack_outputs(results):
    """Sum the 4 TP partials per batch and restore [B, S, D]."""
    out = np.zeros((B, S, D), dtype=np.float32)
    for c, res in enumerate(results):
        b = c // G
        part = np.asarray(res["out"]).astype(np.float32)   # [p, si, col]
        out[b] += part.transpose(1, 0, 2).reshape(S, D)
    return out


def kernel(x, Wqkv, Wo, _trace=False, _trace_kwargs=None):
    from concourse import bass_utils

    nc = _build()
    in_maps = _pack_inputs(x, Wqkv, Wo)
    res = bass_utils.run_bass_kernel_spmd(
        nc, in_maps, core_ids=list(range(8)), trace=_trace,
        **(_trace_kwargs or {}))
    out = _unpack_outputs(res.results)
    if _trace:
        kernel.last_result = res
    return out

